# revision 23
# baseline (speedup 1.0000x reference)
"""AdaptiveConv Trainium2 kernel.

Strategy (data-parallel over batch, one batch element per NeuronCore):
  x[b]: [64, 256, 256] f32, 9 deformable taps with per-pixel bilinear sampling
  followed by a 64x64 channel-mixing matmul per tap, accumulated in PSUM.

Device pipeline per core:
  Phase 0: cast x to bf16 into a zero-padded DRAM image x_pad [64, 262*262+4]
           (3-px zero border makes out-of-range bilinear corners read 0).
  Phase 1: per-pixel coordinate math on DVE in a [128, 512] linear layout
           (partition P holds pixels [P*512, (P+1)*512)): floors, fractional
           weights fy (per tap-row m) / fx (per tap-col n), and per-tap int16
           quad indices relative to a per-strip row window. Results round-trip
           through DRAM so they can be re-read partition-replicated (DRAM
           source APs allow stride-0 dims).
  Phase 2: 32 strips of 8 output rows (2048 px). Per strip: build a bf16
           "quad" tensor Q[c, r, w] = (x[r,w], x[r,w+1], x[r+1,w], x[r+1,w+1])
           for a static 49-row window via 4 strided DMAs; then per tap one
           gpsimd ap_gather (d=4: one index fetches a full 2x2 bilinear patch
           for all 64 channels; partitions 64-127 duplicate the image so the
           8 Q7 cores cover two 1024-px half-strips per instruction), a DVE
           lerp-combine with the replicated fx/fy, and a K=64 matmul per
           half/N-chunk accumulating the 9 taps into PSUM. PSUM DMAs straight
           to the output.

The per-strip row-window bases are baked into the NEFF from the (fixed)
problem inputs; windows are sized for the worst case across all 8 cores so
the SPMD program is identical on every core.
"""
import sys

sys.path.insert(0, "/opt/trn_rl_repo")

import numpy as np

from concourse import bacc, bass, mybir
from concourse import bass_utils
from concourse.tile import TileContext

F32 = mybir.dt.float32
BF16 = mybir.dt.bfloat16
I16 = mybir.dt.int16
I32 = mybir.dt.int32

B, C, H, W = 8, 64, 256, 256
NPIX = H * W
PAD = 3               # zero border width
PH = H + 2 * PAD      # padded height (262)
PW = W + 2 * PAD      # padded width (262)
R_STRIP = 8           # output rows per strip
NSTRIP = H // R_STRIP
SPX = R_STRIP * W     # pixels per strip (2048)
HPX = SPX // 2        # half-strip pixels (1024)
QWIN = 49             # quad rows per strip window
NQ = QWIN * PW        # quad positions per window (12838)
TAPS = 9

_CACHE = {}


def _strip_bases(sy):
    """Static per-strip padded-row window (base, rows), shared across cores."""
    los = []
    for s in range(NSTRIP):
        lo = int(np.floor(sy[:, s * R_STRIP].min())) - 4 + PAD  # padded coords
        lo = max(0, min(lo, PH - (QWIN + 1)))
        hi_need = int(np.floor(sy[:, s * R_STRIP + R_STRIP - 1].max())) + 2 + 1 + PAD
        win = min(max(hi_need - lo + 2, 8), QWIN)
        if hi_need - lo + 1 > QWIN:
            raise RuntimeError(f"strip {s}: window {hi_need - lo + 1} exceeds {QWIN}")
        los.append((lo, win))
    return los


def _build(los):
    nc = bacc.Bacc("TRN2", target_bir_lowering=True)
    x_in = nc.declare_dram_parameter("x", [C, NPIX], F32, isOutput=False)
    sh_in = nc.declare_dram_parameter("sh", [H], F32, isOutput=False)
    sw_in = nc.declare_dram_parameter("sw", [W], F32, isOutput=False)
    dil_in = nc.declare_dram_parameter("dil", [NPIX], F32, isOutput=False)
    wt_in = nc.declare_dram_parameter("wt", [TAPS * C * C], F32, isOutput=False)
    pr_in = nc.declare_dram_parameter("prmap", [NPIX], F32, isOutput=False)
    out = nc.declare_dram_parameter("out", [C, NPIX], F32, isOutput=True)

    x_pad = nc.dram_tensor("x_pad", [C, PH * PW + 4], BF16)
    frac_d = nc.dram_tensor("frac_d", [6, NPIX], BF16)   # fy m=-1,0,1; fx n=-1,0,1
    idx_d = nc.dram_tensor("idx_d", [TAPS, NPIX], I16)

    with TileContext(nc) as tc:
        # ---------------- phase 0 + 1 pool ----------------
        with tc.tile_pool(name="p01", bufs=1) as p01:
            zt = p01.tile([C, 3 * PW], BF16, tag="zt")
            nc.vector.memset(zt[:], 0.0)
            # zero borders of x_pad
            nc.sync.dma_start(out=x_pad[:, 0:3 * PW], in_=zt[:])
            nc.sync.dma_start(out=x_pad[:, (PH - 3) * PW:PH * PW], in_=zt[:])
            nc.sync.dma_start(out=x_pad[:, PH * PW:PH * PW + 4], in_=zt[:, :4])
            lr = x_pad[:, 3 * PW:(PH - 3) * PW].rearrange("c (r w) -> c r w", w=PW)
            nc.sync.dma_start(out=lr[:, :, 0:3], in_=zt[:, :3 * (PH - 6)].rearrange(
                "c (r k) -> c r k", k=3))
            nc.sync.dma_start(out=lr[:, :, PW - 3:PW], in_=zt[:, :3 * (PH - 6)].rearrange(
                "c (r k) -> c r k", k=3))
            # cast interior
            for k in range(16):
                cf = p01.tile([C, 4096], F32, tag="castf")
                cb = p01.tile([C, 4096], BF16, tag="castb")
                nc.sync.dma_start(out=cf[:], in_=x_in[:, k * 4096:(k + 1) * 4096])
                nc.vector.tensor_copy(out=cb[:], in_=cf[:])
                base = (PAD + k * 16) * PW + PAD
                nc.sync.dma_start(
                    out=x_pad[:, base:base + 16 * PW].rearrange(
                        "c (r w) -> c r w", w=PW)[:, :, 0:W],
                    in_=cb[:].rearrange("c (r w) -> c r w", w=W))

            # ---- coordinate math in [128, 512] linear layout ----
            dil_t = p01.tile([128, 512], F32, tag="dil")
            nc.sync.dma_start(out=dil_t[:], in_=dil_in[:].rearrange("(p f) -> p f", p=128))
            sh2 = p01.tile([128, 2], F32, tag="sh2")
            nc.sync.dma_start(out=sh2[:], in_=sh_in[:].rearrange("(p r) -> p r", r=2))
            sy2 = p01.tile([128, 2], F32, tag="sy2")
            nc.vector.tensor_scalar(sy2[:], sh2[:], 127.5, 127.5,
                                    mybir.AluOpType.mult, mybir.AluOpType.add)
            sy_t = p01.tile([128, 512], F32, tag="sy")
            for r in range(2):
                nc.vector.tensor_scalar(sy_t[:, r * W:(r + 1) * W],
                                        dil_t[:, r * W:(r + 1) * W],
                                        0.0, sy2[:, r:r + 1],
                                        mybir.AluOpType.mult, mybir.AluOpType.add)
            sw_t = p01.tile([128, 512], F32, tag="sw")
            nc.sync.dma_start(
                out=sw_t[:].rearrange("p (r w) -> p r w", w=W),
                in_=sw_in[:].rearrange("(a b w) -> a b w", a=1, b=1).broadcast_to((128, 2, W)))
            sx_t = p01.tile([128, 512], F32, tag="sx")
            nc.vector.tensor_scalar(sx_t[:], sw_t[:], 127.5, 127.5,
                                    mybir.AluOpType.mult, mybir.AluOpType.add)

            def floor_frac(base_t, off, tagp):
                """returns (floor_f32_tile, frac_bf16_tile) of base + off*dil"""
                cc = p01.tile([128, 512], F32, tag=tagp + "c")
                if off == 0.0:
                    nc.vector.tensor_copy(out=cc[:], in_=base_t[:])
                else:
                    nc.vector.scalar_tensor_tensor(
                        cc[:], dil_t[:], float(off), base_t[:],
                        mybir.AluOpType.mult, mybir.AluOpType.add)
                ci = p01.tile([128, 512], I32, tag=tagp + "i")
                nc.vector.tensor_copy(out=ci[:], in_=cc[:])
                cf = p01.tile([128, 512], F32, tag=tagp + "f")
                nc.vector.tensor_copy(out=cf[:], in_=ci[:])
                gt = p01.tile([128, 512], F32, tag=tagp + "g")
                nc.vector.tensor_tensor(gt[:], cf[:], cc[:], mybir.AluOpType.is_gt)
                nc.vector.tensor_tensor(cf[:], cf[:], gt[:], mybir.AluOpType.subtract)
                fr = p01.tile([128, 512], F32, tag=tagp + "r")
                nc.vector.tensor_tensor(fr[:], cc[:], cf[:], mybir.AluOpType.subtract)
                fb = p01.tile([128, 512], BF16, tag=tagp + "b")
                nc.vector.tensor_copy(out=fb[:], in_=fr[:])
                return cf, fb

            y0f, x0f = [], []
            for mi, m in enumerate((-1.0, 0.0, 1.0)):
                f0, fb = floor_frac(sy_t, m, f"fy{mi}")
                y0f.append(f0)
                nc.sync.dma_start(out=frac_d[mi, :].rearrange("(p f) -> p f", p=128), in_=fb[:])
            for ni, n in enumerate((-1.0, 0.0, 1.0)):
                f0, fb = floor_frac(sx_t, n, f"fx{ni}")
                x0f.append(f0)
                nc.sync.dma_start(out=frac_d[3 + ni, :].rearrange("(p f) -> p f", p=128), in_=fb[:])

            pr_t = p01.tile([128, 512], F32, tag="pr")
            nc.sync.dma_start(out=pr_t[:], in_=pr_in[:].rearrange("(p f) -> p f", p=128))
            rowmod = []
            for mi in range(3):
                # ring slot = (y0 + PAD) mod QWIN, exact floor-div with correction
                yp = p01.tile([128, 512], F32, tag="yp", name=f"yp{mi}")
                nc.vector.tensor_scalar(yp[:], y0f[mi][:], 1.0, float(PAD),
                                        mybir.AluOpType.mult, mybir.AluOpType.add)
                qq = p01.tile([128, 512], F32, tag="qq", name=f"qq{mi}")
                nc.vector.tensor_scalar_mul(qq[:], yp[:], 1.0 / QWIN)
                qi_ = p01.tile([128, 512], I32, tag="qqi", name=f"qqi{mi}")
                nc.vector.tensor_copy(out=qi_[:], in_=qq[:])
                nc.vector.tensor_copy(out=qq[:], in_=qi_[:])
                q49 = p01.tile([128, 512], F32, tag="q49", name=f"q49{mi}")
                nc.vector.tensor_scalar_mul(q49[:], qq[:], float(QWIN))
                gt_ = p01.tile([128, 512], F32, tag="qgt", name=f"qgt{mi}")
                nc.vector.tensor_tensor(gt_[:], q49[:], yp[:], mybir.AluOpType.is_gt)
                nc.vector.tensor_scalar_mul(gt_[:], gt_[:], float(QWIN))
                nc.vector.tensor_tensor(q49[:], q49[:], gt_[:], mybir.AluOpType.subtract)
                lt_ = p01.tile([128, 512], F32, tag="qlt", name=f"qlt{mi}")
                nc.vector.tensor_scalar(lt_[:], q49[:], 1.0, float(QWIN),
                                        mybir.AluOpType.mult, mybir.AluOpType.add)
                nc.vector.tensor_tensor(lt_[:], lt_[:], yp[:], mybir.AluOpType.is_le)
                nc.vector.tensor_scalar_mul(lt_[:], lt_[:], float(QWIN))
                nc.vector.tensor_tensor(q49[:], q49[:], lt_[:], mybir.AluOpType.add)
                # ring row = yp - q49  (in [0, QWIN))
                rm = p01.tile([128, 512], F32, tag="rm", name=f"rm{mi}")
                nc.vector.tensor_tensor(rm[:], yp[:], q49[:], mybir.AluOpType.subtract)
                rowmod.append(rm)
            for mi in range(3):
                for ni in range(3):
                    tap = mi * 3 + ni
                    qf = p01.tile([128, 512], F32, tag="qf")
                    nc.vector.scalar_tensor_tensor(
                        qf[:], rowmod[mi][:], float(PW), x0f[ni][:],
                        mybir.AluOpType.mult, mybir.AluOpType.add)
                    nc.vector.tensor_scalar_add(qf[:], qf[:], float(PAD))
                    qi = p01.tile([128, 512], I16, tag="qi")
                    nc.vector.tensor_copy(out=qi[:], in_=qf[:])
                    wr = p01.tile([128, 512], I16, tag="wr")
                    nc.vector.tensor_copy(
                        out=wr[:].rearrange("P (p c2) -> P p c2", p=16, c2=32),
                        in_=qi[:].rearrange("P (c2 p) -> P p c2", c2=32, p=16))
                    nc.sync.dma_start(out=idx_d[tap, :].rearrange("(p f) -> p f", p=128), in_=wr[:])

        # ---------------- phase 2 pool ----------------
        with tc.tile_pool(name="p2", bufs=1) as p2, \
             tc.tile_pool(name="p2b", bufs=2) as p2b, \
             tc.tile_pool(name="p2g", bufs=4) as p2g, \
             tc.tile_pool(name="ps", bufs=2, space="PSUM") as ps:
            wt_t = p2.tile([128, TAPS * C], F32, tag="wtf")
            for d2 in range(2):
                nc.sync.dma_start(
                    out=wt_t[d2 * C:(d2 + 1) * C, :].rearrange(
                        "i (t o) -> i t o", t=TAPS),
                    in_=wt_in[:].rearrange("(t i o) -> i t o", t=TAPS, i=C))
            wt_b = p2.tile([128, TAPS * C], BF16, tag="wtb")
            nc.vector.tensor_copy(out=wt_b[:], in_=wt_t[:])

            quad = p2.tile([128, NQ, 4], BF16, tag="quad", name="quad_ring")
            built_hi = 0
            for s in range(NSTRIP):
                lo, win = los[s]
                # rows that must be present: [lo, lo+QWIN); build new ones
                a = max(built_hi, lo)
                b = lo + QWIN
                if s == 0:
                    a = lo
                built_hi = b
                nrows = b - a
                if nrows > 0:
                    xw = p2.tile([128, QWIN * PW + 264], BF16, tag="xw",
                                 name=f"xw_{s}")
                    nxw = nrows * PW + 264
                    for d2 in range(2):
                        nc.sync.dma_start(
                            out=xw[d2 * C:(d2 + 1) * C, :nxw],
                            in_=x_pad[:, a * PW:a * PW + nxw])
                    # ring segments of [a, b) by slot = r % QWIN
                    segs = []
                    r0 = a
                    while r0 < b:
                        sl = r0 % QWIN
                        ln = min(b - r0, QWIN - sl)
                        segs.append((r0 - a, sl, ln))
                        r0 += ln
                    for k, dlt in enumerate((0, 1, PW, PW + 1)):
                        for xoff, sl, ln in segs:
                            dst = quad[:, sl * PW:(sl + ln) * PW, k]
                            srcv = xw[:, xoff * PW + dlt:xoff * PW + dlt + ln * PW]
                            nc.scalar.copy(out=dst, in_=srcv)
                # replicated fx / fy for this strip (parts 0-63: half A, 64-127: B)
                fr_ts = []
                for q in range(6):
                    ft = p2.tile([128, HPX], BF16, tag=f"fr{q}", name=f"fr{q}_{s}")
                    for h in range(2):
                        nc.sync.dma_start(
                            out=ft[h * C:(h + 1) * C, :],
                            in_=frac_d[q, s * SPX + h * HPX:s * SPX + (h + 1) * HPX]
                            .rearrange("(a f) -> a f", a=1).broadcast_to((C, HPX)))
                    fr_ts.append(ft)

                psums = []
                for j in range(4):
                    pst = ps.tile([C, 512], F32, tag=f"ps{j}", name=f"ps{j}_{s}")
                    psums.append(pst)
                for tap in range(TAPS):
                    mi, ni = tap // 3, tap % 3
                    idxt = p2g.tile([128, HPX // 16], I16, tag="idxt")
                    for h in range(2):
                        blk = idx_d[tap, (2 * s + h) * HPX:(2 * s + h + 1) * HPX]
                        for d4 in range(4):
                            nc.sync.dma_start(
                                out=idxt[h * C + d4 * 16:h * C + (d4 + 1) * 16, :]
                                .rearrange("p (ch c2) -> p ch c2", ch=2),
                                in_=blk.rearrange("(ch p c2) -> p ch c2", ch=2, p=16))
                    gout = p2g.tile([128, HPX, 4], BF16, tag="gout")
                    nc.gpsimd.ap_gather(gout[:], quad[:], idxt[:], channels=128,
                                        num_elems=NQ, d=4, num_idxs=HPX)
                    q0 = gout[:, :, 0]
                    q1 = gout[:, :, 1]
                    q2 = gout[:, :, 2]
                    q3 = gout[:, :, 3]
                    fx = fr_ts[3 + ni]
                    fy = fr_ts[mi]
                    t0 = p2b.tile([128, HPX], BF16, tag="t0")
                    u0 = p2b.tile([128, HPX], BF16, tag="u0")
                    nc.vector.tensor_tensor(t0[:], q1, q0, mybir.AluOpType.subtract)
                    nc.vector.tensor_tensor(t0[:], t0[:], fx[:], mybir.AluOpType.mult)
                    nc.vector.tensor_tensor(u0[:], t0[:], q0, mybir.AluOpType.add)
                    t1 = p2b.tile([128, HPX], BF16, tag="t1")
                    u1 = p2b.tile([128, HPX], BF16, tag="u1")
                    nc.vector.tensor_tensor(t1[:], q3, q2, mybir.AluOpType.subtract)
                    nc.vector.tensor_tensor(t1[:], t1[:], fx[:], mybir.AluOpType.mult)
                    nc.vector.tensor_tensor(u1[:], t1[:], q2, mybir.AluOpType.add)
                    samp = p2b.tile([128, HPX], BF16, tag="samp", bufs=3)
                    nc.vector.tensor_tensor(samp[:], u1[:], u0[:], mybir.AluOpType.subtract)
                    nc.vector.tensor_tensor(samp[:], samp[:], fy[:], mybir.AluOpType.mult)
                    nc.vector.tensor_tensor(samp[:], samp[:], u0[:], mybir.AluOpType.add)

                    first, last = tap == 0, tap == TAPS - 1
                    for half in range(2):
                        for chunk in range(2):
                            nc.tensor.matmul(
                                psums[half * 2 + chunk][:],
                                wt_b[half * 64:half * 64 + 64,
                                     tap * C:(tap + 1) * C],
                                samp[half * 64:half * 64 + 64,
                                     chunk * 512:(chunk + 1) * 512],
                                start=first, stop=last)
                for j in range(4):
                    ot = p2b.tile([C, 512], F32, tag="ot", name=f"ot{j}_{s}")
                    nc.scalar.copy(out=ot[:], in_=psums[j][:])
                    nc.sync.dma_start(
                        out=out[:, s * SPX + j * 512:s * SPX + (j + 1) * 512],
                        in_=ot[:])
    nc.finalize()
    return nc


def kernel(x, stride_h, stride_w, dilation, weight):
    x = np.ascontiguousarray(np.asarray(x, dtype=np.float32))
    sh = np.asarray(stride_h, dtype=np.float32)
    sw = np.asarray(stride_w, dtype=np.float32)
    dil = np.asarray(dilation, dtype=np.float32)[:, 0]
    wgt = np.asarray(weight, dtype=np.float32)

    sy = (sh + 1.0) * (H - 1) / 2.0
    los = _strip_bases(sy)
    key = tuple(los)
    if key not in _CACHE:
        _CACHE[key] = _build(los)
    nc = _CACHE[key]

    # host-side layout prep (per-core shards + constant maps)
    wt9 = wgt.transpose(2, 3, 1, 0).reshape(TAPS, C, C)  # [tap, i, o]
    prmap = np.empty(NPIX, np.float32)
    for s in range(NSTRIP):
        prmap[s * SPX:(s + 1) * SPX] = los[s][0] * PW - (PAD * PW + PAD)
    in_maps = []
    for b in range(B):
        in_maps.append({
            "x": x[b].reshape(C, NPIX),
            "sh": sh[b],
            "sw": sw[b],
            "dil": dil[b].reshape(NPIX),
            "wt": np.ascontiguousarray(wt9).reshape(-1),
            "prmap": prmap,
        })
    import os
    trace = bool(os.environ.get("AC_TRACE"))
    res = bass_utils.run_bass_kernel_spmd(nc, in_maps, core_ids=list(range(B)),
                                          trace=trace)
    if trace:
        kernel.last_exec_time_ns = res.exec_time_ns
    outp = np.stack([res.results[b]["out"].reshape(C, H, W) for b in range(B)])
    return outp
